# revision 12
# baseline (speedup 1.0000x reference)
"""Trainium2 Bass kernel for GemNet AtomUpdateBlock (gnn_message_passing).

Computation (per reference):
    bases = basis_rad @ W_rbf              # [E, De]
    x     = m * bases                      # [E, De]
    z     = segment_sum(x, idx_atom, A)    # [A, De]
    x     = silu(z @ W_in)                 # [A, Da]
    3x residual: x = (x + silu(silu(x W1) W2)) / sqrt(2)

Distribution: shard EDGES BY DESTINATION ATOM. Host bins atoms into
8 cores x 40 tiles of <=64 atoms (snake-deal balanced), packs each
tile's edges into ceil(bin_edges/128) columns of 128 edges (variable
per-tile column count, equalized across cores so one program serves
all 8 cores). No collectives: output atom slices are disjoint.

Device pipeline per 128-edge column (all bf16 matmuls, f32 PSUM):
  PE bases: FOUR columns packed per PE pass via 4-quadrant row tiling
      (tile_position (0,0)/(32,0)/(64,0)/(96,0), K=16 each) -- the four
      sub-arrays stream concurrently: ~108ns/col vs 209 for 2-quad.
  mult x = m*bases: three engine routes balanced by a static pattern:
      A: DVE tensor_tensor direct from PSUM (1x wall, ~1.22us/pair)
      B: ACT evac to bf16 + DVE all-SBUF mult (2x, ~0.69us/pair)
      C: ACT evac to bf16 + GpSimd mult (~2.1us/pair)
      (GpSimd cannot read PSUM -- verified; all GP work is SBUF-only.)
  PE scatter: zT[d,a] += x[:,dchunk].T @ S per 128-d-chunk with
      64-atom tiles -> N=64 matmuls (30.8ns measured, weight load
      hides). Two atom tiles share one PSUM bank (interleaved
      accumulation, bank-wide has_written clear on the first matmul);
      one fused ACT copy evacuates the bank pair.
Epilogue per QUAD of 8 tiles (512 atoms, feature-major): bf16 MLP
matmuls at N=512 (res) / N=64 (W_in), silu on ACT, skip-adds as GpSimd
scalar_tensor_tensor with host-folded sqrt2 scaling. Output written
feature-major bf16, untransposed on host.  Edge-stream DMAs ride the
sync queue in 8-column chunks prefetched 3 ahead; weights/outputs ride
the scalar queue.  A warmup matmul burst upclocks the PE.
"""

import math
import os
import sys

import numpy as np
import ml_dtypes

BF16 = ml_dtypes.bfloat16

P = 128
N_CORES = 8
DE, DA, DR, NH = 512, 256, 16, 3
ATILE = 64            # atoms per scatter tile
T_ATOM = 40           # tiles per core; 8 per epilogue quad
CI, CJ = DE // P, DA // P
INV_SQRT_2 = 0.7071067811865476
CHUNK = 8             # columns per edge-stream DMA chunk
PREFETCH = 3          # chunks of DMA lookahead
PEND_DEPTH = 6        # columns buffered between mult issue and scatter
# multiply route pattern: A=DVE-direct-PSUM, B=ACT-evac+DVE-2x,
# C=ACT-evac+GpSimd. Tuned so DVE/ACT/GP all land ~level.
ROUTE = "ACACABACAC"

_NC_CACHE = {}


# ----------------------------------------------------------------------------
# Host-side packing
# ----------------------------------------------------------------------------

def _bin_core_atoms(atom_ids, degs):
    """Pack one core's atoms into T_ATOM bins of <=ATILE atoms, choosing
    each bin's last atom so the edge count lands just under a multiple of
    P (minimizes ceil(edges/P) column padding). Returns list of bins
    (atom-id lists) and their column counts."""
    maxd = int(degs.max()) if len(degs) else 0
    buckets = [[] for _ in range(maxd + 1)]
    for a in atom_ids[::-1]:  # reversed so .pop() yields descending order
        buckets[int(degs[a])].append(a)
    navail = [len(b) for b in buckets]
    rem = len(atom_ids)
    bins = []

    def take(d):
        nonlocal rem
        rem -= 1
        navail[d] -= 1
        return buckets[d].pop()

    hi = maxd
    for b in range(T_ATOM, 0, -1):
        n_b = -(-rem // b)
        assert n_b <= ATILE
        cur = []
        s = 0
        for _ in range(max(0, n_b - 1)):
            while hi > 0 and navail[hi] == 0:
                hi -= 1
            cur.append(take(hi))
            s += hi
        if len(cur) < n_b:
            # tuned last atom: minimal column waste, prefer fuller columns
            best, bkey = None, None
            for d in range(maxd + 1):
                if navail[d] == 0:
                    continue
                key = (-(-(s + d) // P), -((s + d) % P))
                if bkey is None or key < bkey:
                    best, bkey = d, key
            cur.append(take(best))
            s += best
        bins.append((cur, max(1, -(-s // P))))
    assert rem == 0
    return bins


def _pack_layout(idx, n_atoms):
    E = idx.shape[0]
    counts = np.bincount(idx, minlength=n_atoms)

    # snake-deal atoms to cores by degree (balances per-core edge totals)
    order = np.argsort(-counts, kind="stable")
    n_rounds = math.ceil(n_atoms / N_CORES)
    pad = n_rounds * N_CORES - n_atoms
    padded = np.concatenate([order, np.full(pad, -1, dtype=order.dtype)])
    grid = padded.reshape(n_rounds, N_CORES)
    grid[1::2] = grid[1::2, ::-1]
    core_of_atom = np.empty(n_atoms, dtype=np.int64)
    valid = grid >= 0
    core_idx = np.broadcast_to(np.arange(N_CORES), grid.shape)
    core_of_atom[grid[valid]] = core_idx[valid]

    # per-core waste-minimizing binning; bins sorted by K desc => shared
    # rank space across cores
    per_core_sorted = []  # [core][rank] -> (atom_list, K)
    for c in range(N_CORES):
        ids = np.where(core_of_atom == c)[0]
        ids = ids[np.argsort(-counts[ids], kind="stable")]
        bins = _bin_core_atoms(ids, counts)
        bins.sort(key=lambda bk: -bk[1])
        per_core_sorted.append(bins)

    profile0 = np.array(
        [max(per_core_sorted[c][r][1] for c in range(N_CORES))
         for r in range(T_ATOM)],
        dtype=np.int64,
    )
    # deal ranks to epilogue quads snake-wise so each quad of 8 tiles
    # carries a similar column count (keeps epilogues evenly spread)
    n_quads = T_ATOM // 8
    perm = []  # tile t -> rank
    lanes = [[] for _ in range(n_quads)]
    for r in range(T_ATOM):
        q = r % (2 * n_quads)
        lanes[q if q < n_quads else 2 * n_quads - 1 - q].append(r)
    for q in range(n_quads):
        perm += lanes[q]
    perm = np.array(perm, dtype=np.int64)
    profile = profile0[perm]
    padc = (-int(profile.sum())) % 4
    profile[T_ATOM - 1] += padc  # group-align to 4 columns
    NC = int(profile.sum())
    col_off = np.zeros(T_ATOM + 1, dtype=np.int64)
    np.cumsum(profile, out=col_off[1:])

    tile_of_rank = np.empty(T_ATOM, dtype=np.int64)
    tile_of_rank[perm] = np.arange(T_ATOM)

    tile_of_atom = np.empty(n_atoms, dtype=np.int64)
    slot_of_atom = np.empty(n_atoms, dtype=np.int64)
    for c in range(N_CORES):
        for r in range(T_ATOM):
            ids, _k = per_core_sorted[c][r]
            t = tile_of_rank[r]
            tile_of_atom[ids] = t
            slot_of_atom[ids] = np.arange(len(ids))

    ekey = core_of_atom[idx] * T_ATOM + tile_of_atom[idx]
    eorder = np.argsort(ekey, kind="stable")
    ekey_s = ekey[eorder]
    kb = np.bincount(ekey_s, minlength=N_CORES * T_ATOM)
    kb_starts = np.zeros(N_CORES * T_ATOM + 1, dtype=np.int64)
    np.cumsum(kb, out=kb_starts[1:])
    pos_in_bin = np.arange(E) - kb_starts[ekey_s]
    tile_s = ekey_s % T_ATOM
    col_of_edge = col_off[tile_s] + pos_in_bin // P
    assert (col_of_edge < col_off[tile_s + 1]).all()
    prow_of_edge = pos_in_bin % P

    return dict(
        NC=NC,
        profile=tuple(int(v) for v in profile),
        col_off=col_off,
        eorder=eorder,
        core_of_edge=ekey_s // T_ATOM,
        col_of_edge=col_of_edge,
        prow_of_edge=prow_of_edge,
        rel_of_edge=slot_of_atom[idx][eorder],
        core_of_atom=core_of_atom,
        tile_of_atom=tile_of_atom,
        slot_of_atom=slot_of_atom,
    )


def _pack_weights(W_rbf, W_in, res_W1, res_W2):
    Cr = DA // P
    # wrbf duplicated at partition rows 0:16/32:48/64:80/96:112 so four
    # row-tiled bases matmuls share one rhs tile.
    wrbf4 = np.zeros((P, DE), dtype=np.float32)
    for q in range(4):
        wrbf4[32 * q : 32 * q + DR] = W_rbf
    win = W_in.reshape(CI, P, CJ, P).transpose(1, 0, 2, 3).reshape(P, CI * CJ * P)
    blocks = []
    c = INV_SQRT_2
    for l in range(NH):
        w1 = (res_W1[l] * (c ** l)).astype(np.float32)
        w2 = res_W2[l].astype(np.float32)
        for W in (w1, w2):
            blocks.append(
                W.reshape(Cr, P, Cr, P).transpose(1, 0, 2, 3).reshape(P, Cr * Cr * P)
            )
    wres = np.concatenate(blocks, axis=1)
    return (
        np.ascontiguousarray(wrbf4, dtype=BF16),
        np.ascontiguousarray(win, dtype=BF16),
        np.ascontiguousarray(wres, dtype=BF16),
    )


def _build_in_maps(m, basis_rad, layout, W_rbf, W_in, res_W1, res_W2):
    NC = layout["NC"]
    NG = NC // 4
    wrbf4, win, wres = _pack_weights(W_rbf, W_in, res_W1, res_W2)
    m_src = m[layout["eorder"]]
    bas_src = basis_rad[layout["eorder"]]
    core = layout["core_of_edge"]
    col = layout["col_of_edge"]
    prow = layout["prow_of_edge"]
    rel = layout["rel_of_edge"]

    in_maps = []
    for c in range(N_CORES):
        sel = core == c
        cc, pp, rr = col[sel], prow[sel], rel[sel]
        m_pack = np.zeros((P, NC, DE), dtype=BF16)
        m_pack[pp, cc] = m_src[sel].astype(BF16)
        # bases transposed, quadrant-packed: rows 16q:16q+16 hold column
        # 4g+q of group g (DMA'd to SBUF partitions 32q:32q+16)
        bas4 = np.zeros((64, NG * P), dtype=np.float32)
        q = cc % 4
        g = cc // 4
        bas4[(16 * q)[None, :] + np.arange(DR)[:, None], (g * P + pp)[None, :]] = (
            bas_src[sel].T
        )
        s_hot = np.zeros((P, NC * ATILE), dtype=ml_dtypes.float8_e4m3)
        s_hot[pp, cc * ATILE + rr] = 1.0
        in_maps.append(
            dict(
                m_pack=np.ascontiguousarray(m_pack),
                bas4=np.ascontiguousarray(bas4, dtype=BF16),
                s_hot=s_hot,
                wrbf4=wrbf4,
                win=win,
                wres=wres,
            )
        )
    return in_maps


def _unpack_output(results, layout, n_atoms):
    out = np.zeros((n_atoms, DA), dtype=np.float32)
    core_of_atom = layout["core_of_atom"]
    row_of_atom = layout["tile_of_atom"] * ATILE + layout["slot_of_atom"]
    for c in range(N_CORES):
        x = results[c]["out"].astype(np.float32).reshape(P, CJ, T_ATOM * ATILE)
        x_core = x.transpose(2, 1, 0).reshape(T_ATOM * ATILE, DA)
        mask = core_of_atom == c
        out[mask] = x_core[row_of_atom[mask]]
    return out


# ----------------------------------------------------------------------------
# Bass kernel builder
# ----------------------------------------------------------------------------

def _build_nc(profile, NC):
    import concourse.mybir as mybir
    import concourse.tile as tile
    from concourse import bacc

    f32 = mybir.dt.float32
    bf16 = mybir.dt.bfloat16
    f8 = mybir.dt.float8e4
    Cr = DA // P
    C3 = INV_SQRT_2 ** NH
    GAMMA = [float((1.0 / INV_SQRT_2) ** l) for l in range(NH)]
    W4 = 8 * ATILE  # 512 atoms per epilogue quad
    n_quads = T_ATOM // 8
    NG = NC // 4
    n_chunks = math.ceil(NC / CHUNK)

    col_off = [0]
    for k in profile:
        col_off.append(col_off[-1] + k)
    tile_of_col = []
    for t, k in enumerate(profile):
        tile_of_col += [t] * k

    nc = bacc.Bacc(
        "TRN2",
        target_bir_lowering=False,
        debug=False,
        enable_asserts=False,
        num_devices=N_CORES,
    )
    d_m = nc.dram_tensor("m_pack", [P, NC, DE], bf16, kind="ExternalInput")
    d_bas = nc.dram_tensor("bas4", [64, NG * P], bf16, kind="ExternalInput")
    d_s = nc.dram_tensor("s_hot", [P, NC * ATILE], f8, kind="ExternalInput")
    d_wrbf = nc.dram_tensor("wrbf4", [P, DE], bf16, kind="ExternalInput")
    d_win = nc.dram_tensor("win", [P, CI * CJ * P], bf16, kind="ExternalInput")
    d_wres = nc.dram_tensor(
        "wres", [P, NH * 2 * Cr * Cr * P], bf16, kind="ExternalInput"
    )
    d_out = nc.dram_tensor("out", [P, CJ * T_ATOM * ATILE], bf16, kind="ExternalOutput")

    with tile.TileContext(nc) as tc:
        with (
            tc.tile_pool(name="const", bufs=1) as const_p,
            tc.tile_pool(name="bas", bufs=4) as bas_p,
            tc.tile_pool(name="m", bufs=4) as m_p,
            tc.tile_pool(name="s", bufs=4) as s_p,
            tc.tile_pool(name="x", bufs=10) as x_p,
            tc.tile_pool(name="bb", bufs=6) as bb_p,
            tc.tile_pool(name="ztsb", bufs=2) as ztsb_p,
            tc.tile_pool(name="act", bufs=3) as act_p,
            tc.tile_pool(name="outp", bufs=2) as out_p,
            tc.tile_pool(name="psb", bufs=4, space="PSUM") as psb_p,
            tc.tile_pool(name="psz", bufs=2, space="PSUM") as psz_p,
            tc.tile_pool(name="psm", bufs=2, space="PSUM") as psm_p,
        ):
            def emit_silu(out_ap, in_ps_ap):
                nc.scalar.activation(
                    out=out_ap, in_=in_ps_ap,
                    func=mybir.ActivationFunctionType.Silu,
                )

            wrbf_sb = const_p.tile([P, DE], bf16, tag="wrbf")
            nc.sync.dma_start(out=wrbf_sb[:], in_=d_wrbf[:])
            win_sb = const_p.tile([P, CI * CJ * P], bf16, tag="win")
            nc.scalar.dma_start(out=win_sb[:], in_=d_win[:])
            wres_sb = const_p.tile([P, NH * 2 * Cr * Cr * P], bf16, tag="wres")
            nc.scalar.dma_start(out=wres_sb[:], in_=d_wres[:])

            # HAM warmup: dense back-to-back matmuls upclock the PE while
            # the first edge DMAs land.
            warm_in = const_p.tile([P, DE], bf16, tag="warmin")
            nc.gpsimd.memset(warm_in[:], 0.0)
            warm_ps = psb_p.tile([P, DE], f32, space="PSUM", tag="bases",
                                 name="warm")
            for w in range(16):
                nc.tensor.matmul(
                    out=warm_ps[:],
                    lhsT=warm_in[:, (w % 4) * P : (w % 4 + 1) * P],
                    rhs=warm_in[:],
                    start=True,
                    stop=True,
                )

            def issue_chunk_dmas(ch):
                c0 = ch * CHUNK
                w = min(CHUNK, NC - c0)
                g0 = c0 // 4
                gw = (w + 3) // 4
                bas_sb = bas_p.tile([P, 2, P], bf16, tag="bas", name=f"bas{ch}")
                for q in range(4):
                    nc.sync.dma_start(
                        out=bas_sb[32 * q : 32 * q + 16, 0:gw, :],
                        in_=d_bas[16 * q : 16 * q + 16, g0 * P : (g0 + gw) * P],
                    )
                m_t = m_p.tile([P, CHUNK, DE], bf16, tag="m", name=f"m{ch}")
                s_t = s_p.tile([P, CHUNK, ATILE], f8, tag="s", name=f"s{ch}")
                if ch == 0:
                    # land the first pair + s early so compute starts sooner
                    nc.sync.dma_start(out=m_t[:, 0:2, :], in_=d_m[:, 0:2, :])
                    nc.sync.dma_start(
                        out=s_t[:, 0:w, :],
                        in_=d_s[:, c0 * ATILE : (c0 + w) * ATILE],
                    )
                    nc.sync.dma_start(out=m_t[:, 2:w, :], in_=d_m[:, 2:w, :])
                else:
                    nc.sync.dma_start(
                        out=m_t[:, 0:w, :], in_=d_m[:, c0 : c0 + w, :]
                    )
                    nc.sync.dma_start(
                        out=s_t[:, 0:w, :],
                        in_=d_s[:, c0 * ATILE : (c0 + w) * ATILE],
                    )
                return (bas_sb, m_t, s_t)

            def epilogue_gen(q, zt_sb, fill=False):
                """Quad epilogue, emitted as units interleaved into the next
                quad's scatter stream. fill=True pads latency boundaries of
                the final quad with dummy matmuls to hold the PE clock."""
                fill_t = [None]

                def pad(n):
                    if not fill:
                        return
                    if fill_t[0] is None:
                        fill_t[0] = psb_p.tile(
                            [P, DE], f32, space="PSUM", tag="bases",
                            name="tailfill"
                        )
                    for w in range(n):
                        nc.tensor.matmul(
                            out=fill_t[0][:],
                            lhsT=warm_in[:, (w % 4) * P : (w % 4 + 1) * P],
                            rhs=warm_in[:],
                            start=True,
                            stop=True,
                        )

                X = act_p.tile([P, Cr * W4], bf16, tag="X", name=f"X{q}_0")
                for j in range(CJ):
                    u_j = psm_p.tile(
                        [P, W4], f32, space="PSUM", tag="misc", name=f"ups{q}_{j}"
                    )
                    for sub in range(4):  # pairs of 64-atom subtiles: N=128
                        for c in range(CI):
                            fi = c * CJ + j
                            nc.tensor.matmul(
                                out=u_j[:, sub * P : (sub + 1) * P],
                                lhsT=win_sb[:, fi * P : (fi + 1) * P],
                                rhs=zt_sb[:, c, 2 * sub : 2 * sub + 2, :],
                                start=(sub == 0 and c == 0),
                                stop=(sub == 3 and c == CI - 1),
                                skip_group_check=True,
                            )
                            if c % 2 == 1:
                                yield
                    emit_silu(X[:, j * W4 : (j + 1) * W4], u_j[:])
                pad(4)
                yield
                for l in range(NH):
                    u1 = act_p.tile([P, Cr * W4], bf16, tag="u1",
                                    name=f"u1_{q}_{l}")
                    for j in range(Cr):
                        v_j = psm_p.tile(
                            [P, W4], f32, space="PSUM", tag="misc",
                            name=f"vps{q}_{l}_{j}"
                        )
                        for i in range(Cr):
                            fi = ((l * 2 + 0) * Cr + i) * Cr + j
                            nc.tensor.matmul(
                                out=v_j[:],
                                lhsT=wres_sb[:, fi * P : (fi + 1) * P],
                                rhs=X[:, i * W4 : (i + 1) * W4],
                                start=(i == 0),
                                stop=(i == Cr - 1),
                            )
                            yield
                        emit_silu(u1[:, j * W4 : (j + 1) * W4], v_j[:])
                    pad(3)
                    yield
                    Y = act_p.tile([P, Cr * W4], bf16, tag="y", name=f"Y{q}_{l}")
                    for j in range(Cr):
                        w_j = psm_p.tile(
                            [P, W4], f32, space="PSUM", tag="misc",
                            name=f"wps{q}_{l}_{j}"
                        )
                        for i in range(Cr):
                            fi = ((l * 2 + 1) * Cr + i) * Cr + j
                            nc.tensor.matmul(
                                out=w_j[:],
                                lhsT=wres_sb[:, fi * P : (fi + 1) * P],
                                rhs=u1[:, i * W4 : (i + 1) * W4],
                                start=(i == 0),
                                stop=(i == Cr - 1),
                            )
                            yield
                        emit_silu(Y[:, j * W4 : (j + 1) * W4], w_j[:])
                    pad(3)
                    yield
                    Xn = act_p.tile(
                        [P, Cr * W4], bf16, tag="X", name=f"X{q}_{l + 1}"
                    )
                    nc.vector.scalar_tensor_tensor(
                        out=Xn[:],
                        in0=Y[:],
                        scalar=GAMMA[l],
                        in1=X[:],
                        op0=mybir.AluOpType.mult,
                        op1=mybir.AluOpType.add,
                    )
                    X = Xn
                    pad(4)
                    yield
                o_t = out_p.tile([P, CJ * W4], bf16, tag="out")
                nc.vector.tensor_scalar(
                    out=o_t[:], in0=X[:], scalar1=float(C3), scalar2=None,
                    op0=mybir.AluOpType.mult,
                )
                for j in range(CJ):
                    nc.scalar.dma_start(
                        out=d_out[
                            :,
                            j * T_ATOM * ATILE + q * W4 : j * T_ATOM * ATILE
                            + (q + 1) * W4,
                        ],
                        in_=o_t[:, j * W4 : (j + 1) * W4],
                    )
                yield

            # ---------------- main column stream ----------------
            tiles = {}
            for ch in range(min(PREFETCH, n_chunks)):
                tiles[ch] = issue_chunk_dmas(ch)

            state = dict(prev_epi=None, dmy=None, dmy_n=0, zt_ps=None,
                         zt_sb=None, quad_done=False)
            pend = []

            def step_epi():
                if state["prev_epi"] is not None:
                    next(state["prev_epi"], None)
                else:
                    w = state["dmy_n"]
                    state["dmy_n"] += 1
                    if w % 2 == 1:
                        return
                    if state["dmy"] is None:
                        state["dmy"] = psm_p.tile(
                            [P, W4], f32, space="PSUM", tag="misc", name="dmy"
                        )
                    nc.tensor.matmul(
                        out=state["dmy"][:],
                        lhsT=warm_in[:, (w % 4) * P : (w % 4 + 1) * P],
                        rhs=warm_in[:],
                        start=True,
                        stop=True,
                    )

            def scatter_col(col, x_t, s_t):
                t = tile_of_col[col]
                u = t // 2
                if col == col_off[2 * u]:
                    state["zt_ps"] = psz_p.tile(
                        [P, CI, 2, ATILE], f32, space="PSUM", tag="z",
                        name=f"ztps{u}"
                    )
                    if u % 4 == 0:
                        state["zt_sb"] = ztsb_p.tile(
                            [P, CI, 8, ATILE], bf16, tag="ztsb",
                            name=f"ztsb{u // 4}"
                        )
                zt_ps = state["zt_ps"]
                first = col == col_off[2 * u]
                last = col == col_off[2 * u + 2] - 1
                for ci in range(CI):
                    nc.tensor.matmul(
                        out=zt_ps[:, ci, t % 2, :],
                        lhsT=x_t[:, ci * P : (ci + 1) * P],
                        rhs=s_t[:, col % CHUNK, :],
                        start=(first and ci == 0),
                        stop=(last and ci == CI - 1),
                        skip_group_check=True,
                    )
                if last:
                    v = u % 4
                    # alternate evac engine to balance ACT/DVE
                    if u % 2 == 0:
                        nc.scalar.copy(
                            out=state["zt_sb"][:, :, 2 * v : 2 * v + 2, :],
                            in_=zt_ps[:],
                        )
                    else:
                        nc.vector.tensor_copy(
                            out=state["zt_sb"][:, :, 2 * v : 2 * v + 2, :],
                            in_=zt_ps[:],
                        )
                    if v == 3:
                        state["quad_done"] = True

            def emit_mult(psb_t, m_t, k0, route, name):
                x_t = x_p.tile([P, DE], bf16, tag="x", name=name)
                if route == "A":
                    nc.vector.tensor_tensor(
                        out=x_t[:],
                        in0=psb_t[:],
                        in1=m_t[:, k0, :],
                        op=mybir.AluOpType.mult,
                    )
                else:
                    bb = bb_p.tile([P, DE], bf16, tag="bb", name=f"bb{name}")
                    nc.scalar.copy(out=bb[:], in_=psb_t[:])
                    eng = nc.vector if route == "B" else nc.gpsimd
                    eng.tensor_tensor(
                        out=x_t[:],
                        in0=bb[:],
                        in1=m_t[:, k0, :],
                        op=mybir.AluOpType.mult,
                    )
                return x_t

            group_tiles = [None, None, None, None]
            quad_idx = [0]
            for c in range(NC):
                ch = c // CHUNK
                if c % CHUNK == 0:
                    chf = ch + PREFETCH
                    if chf < n_chunks:
                        tiles[chf] = issue_chunk_dmas(chf)
                bas_sb, m_t, s_t = tiles[ch]
                if c % 4 == 0:
                    # bases quad: cols c..c+3, four concurrent quadrants
                    gpar = (c // 4) % 2
                    for q in range(4):
                        pt = psb_p.tile([P, DE], f32, space="PSUM",
                                        tag="bases", name=f"bps{c + q}")
                        group_tiles[q] = pt
                        nc.tensor.matmul(
                            out=pt[:],
                            lhsT=bas_sb[32 * q : 32 * q + 16, gpar, :],
                            rhs=wrbf_sb[32 * q : 32 * q + 16, :],
                            start=True,
                            stop=True,
                            tile_position=(32 * q, 0),
                        )
                psb_t = group_tiles[c % 4]
                route = ROUTE[c % len(ROUTE)]
                x_t = emit_mult(psb_t, m_t, c % CHUNK, route, f"x{c}")
                pend.append((x_t, c, s_t))
                step_epi()
                while len(pend) > PEND_DEPTH:
                    xx, cc, ss = pend.pop(0)
                    scatter_col(cc, xx, ss)
                if state["quad_done"]:
                    state["quad_done"] = False
                    if state["prev_epi"] is not None:
                        for _ in state["prev_epi"]:
                            pass
                    state["prev_epi"] = epilogue_gen(
                        quad_idx[0], state["zt_sb"],
                        fill=(quad_idx[0] == n_quads - 1),
                    )
                    quad_idx[0] += 1

            for xx, cc, ss in pend:
                scatter_col(cc, xx, ss)
            if state["quad_done"]:
                state["quad_done"] = False
                if state["prev_epi"] is not None:
                    for _ in state["prev_epi"]:
                        pass
                state["prev_epi"] = epilogue_gen(
                    quad_idx[0], state["zt_sb"], fill=True
                )
                quad_idx[0] += 1
            if state["prev_epi"] is not None:
                for _ in state["prev_epi"]:
                    pass
            assert quad_idx[0] == n_quads, (quad_idx[0], n_quads)

    nc.compile()
    return nc


def _get_nc(profile, NC):
    key = (profile, NC)
    if key not in _NC_CACHE:
        _NC_CACHE[key] = _build_nc(profile, NC)
    return _NC_CACHE[key]


# ----------------------------------------------------------------------------
# Entry point
# ----------------------------------------------------------------------------

def kernel(h, m, basis_rad, idx_atom, W_rbf, W_in, res_W1, res_W2):
    from concourse.bass_utils import run_bass_kernel_spmd

    m = np.asarray(m, dtype=np.float32)
    basis_rad = np.asarray(basis_rad, dtype=np.float32)
    idx = np.asarray(idx_atom).astype(np.int64)
    W_rbf = np.asarray(W_rbf, dtype=np.float32)
    W_in = np.asarray(W_in, dtype=np.float32)
    res_W1 = np.asarray(res_W1, dtype=np.float32)
    res_W2 = np.asarray(res_W2, dtype=np.float32)
    n_atoms = np.asarray(h).shape[0]

    layout = _pack_layout(idx, n_atoms)
    in_maps = _build_in_maps(m, basis_rad, layout, W_rbf, W_in, res_W1, res_W2)
    nc = _get_nc(layout["profile"], layout["NC"])

    trace = os.environ.get("KERNEL_TRACE", "0") == "1"
    res = run_bass_kernel_spmd(
        nc, in_maps, core_ids=list(range(N_CORES)), trace=trace
    )
    if trace and res.exec_time_ns is not None:
        print(f"HW exec time: {res.exec_time_ns} ns", file=sys.stderr)
        kernel.last_exec_time_ns = res.exec_time_ns
    kernel.last_results = res
    return _unpack_output(res.results, layout, n_atoms)
